# revision 1
# baseline (speedup 1.0000x reference)
"""MoE-LoRA layer (nn_MoELoRALayer) as a Bass/Tile kernel for 8 Trainium2 cores.

Computation (per token n):
    logits = x @ W_router.T                    # [N, 8]
    combine = renorm(top2(softmax(logits)))    # [N, 8]
    h       = x @ A_cat.T                      # [N, 128]   (8 experts x rank 16)
    hw      = h * combine_expanded             # [N, 128]
    out     = x @ W_base.T + b + 2.0 * hw @ B_cat.T

Sharding: data-parallel over tokens (1024 per core), all weights replicated.
Matmul operands bf16 (cast host-side), fp32 PSUM accumulation, fp32 output.

Structure per core (v2 — PE-gap-free schedule):
  phase 1: one fused matmul per (k, n): moving = [A^T tile | W_router^T tile]
    (136 cols), stationary = x tile, so h and the router logits land
    TOKEN-MAJOR in one PSUM slot and the router costs 8 extra columns
    instead of separate narrow matmuls. The 8 accumulators pack 3-per-bank
    (column offsets 0/168/336 in 3 banks), leaving 5 banks free so phase 2
    starts with zero drain wait.
  routing: top-2 renormalized softmax runs on DVE/ACT directly in token-major
    layout (no logits transposes), batched 3 token-tiles per op via strided
    APs; combine expansion over ranks is a stride-0 broadcast AP multiply.
    One PE transpose per token tile turns hw into the j-major LoRA stationary;
    transposes are sprinkled between phase-2 matmuls so the PE never waits.
  phase 2: 16 half-sweeps (o-tile x 4 token-tiles, full K), alternating two
    4-bank PSUM sets so drains overlap the next sweep. W_base.T streams once
    into a double-buffered SBUF residency (o-granular, 8-k-tile chunk DMAs);
    the LoRA up-projection is appended per accumulator before stop, giving
    the routing chain a whole sweep of slack. Bias adds on DVE, DMA out.

Host-side layout prep (part of sharding):
    xt    [32, 128, 1024] = x_shard.T, K-tile major (contraction on partitions)
    wto   [8, 4096, 512]  = W_base.T packed per 512-wide output tile
    arhm  [128, 32, 136]  = per-K-tile [A^T | W_router^T] fused moving tensor
    bft   [128, 4096]     = 2.0 * B.transpose(0,2,1).reshape(128, 4096)
    identb [128, 128]     = bf16 identity for PE transposes
"""

import numpy as np

import concourse.bacc as bacc
import concourse.bass as bass
import concourse.mybir as mybir
import concourse.tile as tile
from concourse.bass_utils import run_bass_kernel_spmd

N_CORES = 8
D_IN = 4096
D_OUT = 4096
N_EXP = 8
R = 16
J = N_EXP * R           # 128
SCALING = 2.0
TOK = 1024              # tokens per core
K_TILES = D_IN // 128   # 32
N_TILES = TOK // 128    # 8
O_TILES = D_OUT // 512  # 8
ARH = J + N_EXP         # 136 fused h+router columns
SLOT = 168              # column pitch of arh slots inside a PSUM bank

F32 = mybir.dt.float32
BF16 = mybir.dt.bfloat16

_CACHE = {}


def _build_program(finalize=True):
    key = ("nc", finalize)
    if key in _CACHE:
        return _CACHE[key]

    nc = bacc.Bacc(trn_type="TRN2")

    # Both streams are stored pre-transposed (partition dim first) so every
    # chunk DMA is a straight contiguous copy with 8KB-per-partition lines —
    # the DMA engine pool is descriptor-processing-bound, so line size sets
    # the effective stream bandwidth.
    xt_d = nc.dram_tensor("xt", [8, 128, 4, TOK], BF16, kind="ExternalInput")
    wto_d = nc.dram_tensor("wto", [O_TILES, 128, K_TILES, 512], BF16,
                           kind="ExternalInput")
    arhm_d = nc.dram_tensor("arhm", [128, K_TILES, ARH], BF16, kind="ExternalInput")
    bft_d = nc.dram_tensor("bft", [J, D_OUT], BF16, kind="ExternalInput")
    bvec_d = nc.dram_tensor("bvec", [D_OUT], F32, kind="ExternalInput")
    idb_d = nc.dram_tensor("identb", [128, 128], BF16, kind="ExternalInput")
    out_d = nc.dram_tensor("out", [TOK, D_OUT], F32, kind="ExternalOutput")

    out_ap = out_d[:]
    mm = nc.tensor.matmul
    X = mybir.AxisListType.X
    OP = mybir.AluOpType

    with tile.TileContext(nc) as tc:
        with (
            tc.tile_pool(name="xt_pool", bufs=8) as xt_pool,
            tc.tile_pool(name="res", bufs=1) as res,
            tc.tile_pool(name="outp", bufs=8) as outp,
            tc.tile_pool(name="rsm", bufs=2) as rsm,
            tc.tile_pool(name="ps", bufs=8, space="PSUM") as ps,
        ):
            # ---- DMA schedule ----
            # Per-queue DMA bandwidth is ~1/3 of the core's HBM share, so the
            # start-critical transfers (arhm, xt, W o0 — everything phase 1 +
            # the scheduler-interleaved first sweep touch, in k-need order)
            # are greedily striped across all three queues by byte load.
            # bias/W o1+ are deferred behind them.
            arhm_sb = res.tile([128, K_TILES, ARH], BF16)
            idb_sb = res.tile([128, 128], BF16)

            xts = []
            xtiles = []
            for c in range(8):
                t = xt_pool.tile([128, 4, TOK], BF16, tag="xt", name=f"xt_{c}")
                xtiles.append(t)
                for kk in range(4):
                    xts.append(t[:, kk, :])

            # W_base.T double-buffer: o -> wsb[o % 2], 4 chunk-DMAs of 8
            # K-tiles each (own semaphores => K-ordered consumption). Tile
            # inserts deps in EMISSION order, so prefetches for o >= 2 are
            # emitted inside the sweep loop, after the previous occupant's
            # readers exist (otherwise the DMA gets no WAR wait and clobbers
            # W mid-sweep).
            wsb = [
                res.tile([128, K_TILES, 512], BF16, name=f"wsb{i}")
                for i in range(2)
            ]

            bias_sb = res.tile([128, D_OUT], F32)
            bft_sb = res.tile([J, D_OUT], BF16)

            def w_chunk(o, cc, q, kt=8):
                q.dma_start(
                    out=wsb[o % 2][:, cc * kt:(cc + 1) * kt, :],
                    in_=wto_d[o, :, cc * kt:(cc + 1) * kt, :],
                )

            def issue_w(o):
                for cc in range(4):
                    w_chunk(o, cc, nc.sync if cc % 2 == 0 else nc.scalar)

            # Critical items in k-need order: (bytes, emit_fn). Greedy
            # min-load assignment to the three queues keeps each queue's
            # items in need order while balancing arrival time.
            def arhm_chunk(cc, q):
                q.dma_start(
                    out=arhm_sb[:, cc * 8:(cc + 1) * 8, :],
                    in_=arhm_d[:, cc * 8:(cc + 1) * 8, :],
                )

            def xt_half(c, h, q):
                q.dma_start(
                    out=xtiles[c][:, h * 2:(h + 1) * 2, :],
                    in_=xt_d[c, :, h * 2:(h + 1) * 2, :],
                )

            # Critical stream in strict k-need order at ~0.5MB granularity,
            # round-robined across the three queues so per-queue arrival
            # tracks the consumption frontier.
            crit = []
            for c in range(8):
                if c % 2 == 0:
                    crit.append(lambda q, cc=c // 2: arhm_chunk(cc, q))
                crit.append(lambda q, cc=c, h=0: xt_half(cc, h, q))
                crit.append(lambda q, cc=c, h=1: xt_half(cc, h, q))
                crit.append(lambda q, cc=c: w_chunk(0, cc, q, kt=4))
                if c == 3:
                    crit.append(
                        lambda q: q.dma_start(out=idb_sb, in_=idb_d[:])
                    )
            queues3 = [nc.sync, nc.scalar, nc.gpsimd]
            for idx, emit in enumerate(crit):
                emit(queues3[idx % 3])
            # Post-stream transfers, in deadline order, behind the stripe on
            # the HW queues: W o1 (sweep 2), bft (sweep 1's LoRA + merge),
            # bias o0-slice (merge + sweep-1 drains), rest of bias.
            w_chunk(1, 0, nc.sync)
            w_chunk(1, 1, nc.scalar)
            nc.sync.dma_start(out=bft_sb, in_=bft_d[:])
            w_chunk(1, 2, nc.sync)
            nc.scalar.dma_start(
                out=bias_sb[:, 0:512],
                in_=bvec_d[0:512].partition_broadcast(128),
            )
            w_chunk(1, 3, nc.scalar)
            nc.scalar.dma_start(
                out=bias_sb[:, 512:D_OUT],
                in_=bvec_d[512:D_OUT].partition_broadcast(128),
            )
            # bias arrives on a SWDGE queue; observe it on the DVE clock once.
            btch = rsm.tile([1, 1], F32, tag="btch")
            nc.vector.tensor_copy(out=btch, in_=bias_sb[0:1, 0:1])

            hwt_sb = res.tile([J, TOK], BF16)

            # ---- phase 1: fused h + router logits, token-major ----
            # 8 accumulators packed 3-per-bank => banks 0-2; bank 3 is the
            # transpose scratch; banks 4-7 stay free for the first half-sweep.
            # A matmul with start=True zeroes its WHOLE PSUM bank, which would
            # erase sibling slots' first contribution — so memset the banks
            # once and accumulate with start=False throughout.
            P = [
                ps.tile([128, 512], F32, tag="ps", name=f"arhP{i}")
                for i in range(3)
            ]
            for i in range(3):
                nc.vector.memset(P[i], 0.0)

            def arh_ap(n):
                i, s = divmod(n, 3)
                return P[i][:, s * SLOT:s * SLOT + ARH]

            for k in range(K_TILES):
                for n in range(N_TILES):
                    mm(arh_ap(n), xts[k][:, n * 128:(n + 1) * 128],
                       arhm_sb[:, k, :], start=False, stop=(k == K_TILES - 1),
                       skip_group_check=True)

            pt = ps.tile([128, 128], BF16, tag="ps", name="pt")

            # ---- routing: top-2 renormalized softmax, token-major per n ----
            #   m1 = max_e l; t = l - m1; m2 = max_e (t | top1 -> -inf)
            #   combine_e = [t >= m2] * exp(t) / (1 + exp(m2))
            hw_sbs = []

            def emit_routing_n(n):
                i, s = divmod(n, 3)
                lg = P[i][:, s * SLOT + J:s * SLOT + ARH]     # [128, 8] PSUM
                m1 = rsm.tile([128, 1], F32, tag="m1", bufs=4)
                nc.vector.tensor_reduce(m1, lg, axis=X, op=OP.max)
                t = rsm.tile([128, N_EXP], F32, tag="t", bufs=4)
                nc.vector.tensor_scalar(
                    out=t, in0=lg, scalar1=m1, scalar2=None,
                    op0=OP.subtract,
                )
                eq = rsm.tile([128, N_EXP], F32, tag="eq", bufs=4)
                nc.vector.tensor_scalar(
                    out=eq, in0=t, scalar1=0.0, scalar2=None, op0=OP.is_ge
                )
                msk = rsm.tile([128, N_EXP], F32, tag="msk", bufs=4)
                nc.vector.scalar_tensor_tensor(
                    out=msk, in0=eq, scalar=-1e30, in1=t,
                    op0=OP.mult, op1=OP.add,
                )
                m2 = rsm.tile([128, 1], F32, tag="m2", bufs=4)
                nc.vector.tensor_reduce(m2, msk, axis=X, op=OP.max)
                e2 = rsm.tile([128, 1], F32, tag="e2", bufs=4)
                nc.scalar.activation(e2, m2, mybir.ActivationFunctionType.Exp)
                den = rsm.tile([128, 1], F32, tag="den", bufs=4)
                nc.vector.tensor_scalar_add(den, e2, 1.0)
                rec = rsm.tile([128, 1], F32, tag="rec", bufs=4)
                nc.vector.reciprocal(rec, den)
                et = rsm.tile([128, N_EXP], F32, tag="et", bufs=4)
                nc.scalar.activation(et, t, mybir.ActivationFunctionType.Exp)
                ge = rsm.tile([128, N_EXP], F32, tag="ge", bufs=4)
                nc.vector.tensor_scalar(
                    out=ge, in0=t, scalar1=m2, scalar2=None, op0=OP.is_ge
                )
                w = rsm.tile([128, N_EXP], F32, tag="w", bufs=4)
                nc.vector.tensor_tensor(out=w, in0=et, in1=ge, op=OP.mult)
                cmb = rsm.tile([128, N_EXP], F32, tag="cmb",
                               name=f"cmb_{n}", bufs=8)
                nc.vector.tensor_scalar_mul(cmb, w, rec)
                hw = rsm.tile([128, N_EXP, R], BF16, tag="hw",
                              name=f"hw_{n}", bufs=8)
                nc.vector.tensor_tensor(
                    out=hw,
                    in0=P[i][:, s * SLOT:s * SLOT + J].rearrange(
                        "p (e r) -> p e r", r=R
                    ),
                    in1=cmb.broadcast_to([128, N_EXP, R]),
                    op=OP.mult,
                )
                hw_sbs.append(hw)

            def emit_routing():
                for n in range(N_TILES):
                    emit_routing_n(n)

            def emit_tpose(n):
                # hw (token-major) -> hwt column block (j-major stationary)
                nc.tensor.transpose(
                    out=pt, in_=hw_sbs[n].rearrange("p e r -> p (e r)"),
                    identity=idb_sb,
                )
                nc.vector.tensor_copy(
                    out=hwt_sb[:, n * 128:(n + 1) * 128], in_=pt
                )

            def dummy(tag):
                # claims a PSUM pool slot; the [1,1] memset chains the
                # slot's WAR dependencies through to the next occupant.
                d = ps.tile([128, 512], F32, tag="ps", name=f"dummy_{tag}")
                nc.vector.memset(d[0:1, 0:1], 0.0)

            outq = [nc.scalar, nc.sync, nc.gpsimd]
            osl0 = slice(0, 512)

            # ---- phase 2: 16 half-sweeps over (o-tile, 4 token-tiles) ----
            # Sweep 0 (o0, n0-3) runs k-interleaved with phase 1 during the
            # xt stream. It carries NO LoRA matmul: its plain copy-drains are
            # emitted BEFORE the routing chain, so the DVE frees banks 4-7
            # within ~2us of the last xt chunk and sweep 1 starts
            # immediately (instead of waiting ~15us for routing to release
            # the phase-1 banks). Sweep 0's LoRA term is merged afterwards
            # through two scratch banks once routing has produced hwt.
            s0_accs = [
                ps.tile([128, 512], F32, tag="ps", name=f"acc0_{i}")
                for i in range(4)
            ]
            for k in range(K_TILES):
                for i in range(4):
                    mm(s0_accs[i], xts[k][:, i * 128:(i + 1) * 128],
                       wsb[0][:, k, :], start=(k == 0),
                       stop=(k == K_TILES - 1))
            s0_osbs = []
            for i in range(4):
                osb = outp.tile([128, 512], F32, tag="ob", name=f"ob0_{i}")
                nc.vector.tensor_copy(out=osb, in_=s0_accs[i])
                s0_osbs.append(osb)

            emit_routing()

            for i in range(4):
                dummy(f"a{i}")   # slots 0-3 (phase-1 banks), chained

            # sweep 1: (o0, n4-7) on banks 4-7, LoRA appended last.
            s1_accs = [
                ps.tile([128, 512], F32, tag="ps", name=f"acc1_{i}")
                for i in range(4)
            ]
            for k in range(K_TILES):
                if k >= 2 and (k - 2) % 3 == 0 and (k - 2) // 3 < 8:
                    emit_tpose((k - 2) // 3)
                for i, n in enumerate(range(4, 8)):
                    mm(s1_accs[i], xts[k][:, n * 128:(n + 1) * 128],
                       wsb[0][:, k, :], start=(k == 0), stop=False)
            for i, n in enumerate(range(4, 8)):
                mm(s1_accs[i], hwt_sb[:, n * 128:(n + 1) * 128],
                   bft_sb[:, osl0], start=False, stop=True)

            # sweep 0's deferred LoRA: scratch banks 0/1, merged into the
            # held-back copy-drains (+ bias), then out.
            L = [ps.tile([128, 512], F32, tag="ps", name=f"lmerge{i}")
                 for i in range(2)]
            for i in range(4):
                mm(L[i % 2], hwt_sb[:, i * 128:(i + 1) * 128],
                   bft_sb[:, osl0], start=True, stop=True)
                ta = outp.tile([128, 512], F32, tag="ob", name=f"obm{i}")
                nc.vector.tensor_tensor(
                    out=ta, in0=s0_osbs[i], in1=L[i % 2], op=OP.add
                )
                tb = outp.tile([128, 512], F32, tag="ob", name=f"obn{i}")
                nc.vector.tensor_tensor(
                    out=tb, in0=ta, in1=bias_sb[:, osl0], op=OP.add
                )
                outq[i % 3].dma_start(
                    out=out_ap[i * 128:(i + 1) * 128, osl0], in_=tb
                )
            for i in range(2):
                dummy(f"b{i}")   # slots 2-3 behind the L banks

            # sweep 1 drains (bias has long arrived).
            for i, n in enumerate(range(4, 8)):
                osb = outp.tile([128, 512], F32, tag="ob", name=f"ob1_{i}")
                nc.vector.tensor_tensor(
                    out=osb, in0=s1_accs[i], in1=bias_sb[:, osl0], op=OP.add
                )
                outq[i % 3].dma_start(
                    out=out_ap[n * 128:(n + 1) * 128, osl0], in_=osb
                )
            for i in range(4):
                dummy(f"c{i}")   # slots 4-7, realigns sweep 2 onto 0-3

            for sw in range(2, 2 * O_TILES):
                o, half = divmod(sw, 2)
                if half == 0 and o + 1 < O_TILES:
                    issue_w(o + 1)   # wsb[(o+1)%2]'s readers (o-1) are emitted
                nset = [half * 4 + i for i in range(4)]
                osl = slice(o * 512, (o + 1) * 512)
                accs = [
                    ps.tile([128, 512], F32, tag="ps", name=f"acc{sw}_{i}")
                    for i in range(4)
                ]
                # LoRA opens the group so each accumulator's drain follows
                # its k=31 matmul immediately.
                for i, n in enumerate(nset):
                    mm(accs[i], hwt_sb[:, n * 128:(n + 1) * 128],
                       bft_sb[:, osl], start=True, stop=False)
                for k in range(K_TILES):
                    for i, n in enumerate(nset):
                        mm(accs[i], xts[k][:, n * 128:(n + 1) * 128],
                           wsb[o % 2][:, k, :], start=False,
                           stop=(k == K_TILES - 1))
                last = sw == 2 * O_TILES - 1
                for i, n in enumerate(nset):
                    osb = outp.tile([128, 512], F32, tag="ob",
                                    name=f"ob{sw}_{i}")
                    nc.vector.tensor_tensor(
                        out=osb, in0=accs[i], in1=bias_sb[:, osl], op=OP.add
                    )
                    if last:
                        # split the final transfers across queues so the
                        # kernel's tail is not one serialized out-DMA chain
                        for h in range(2):
                            outq[(i * 2 + h) % 3].dma_start(
                                out=out_ap[n * 128:(n + 1) * 128,
                                           o * 512 + h * 256:
                                           o * 512 + (h + 1) * 256],
                                in_=osb[:, h * 256:(h + 1) * 256],
                            )
                    else:
                        outq[(sw * 4 + i) % 3].dma_start(
                            out=out_ap[n * 128:(n + 1) * 128, osl], in_=osb
                        )

    if finalize:
        nc.finalize()
    _CACHE[key] = nc
    return nc


def _prep_inputs(x, W_base, b_base, W_router, A, B):
    """Shard + lay out inputs for the 8 cores. Returns list of in_maps."""
    import ml_dtypes
    bf16 = ml_dtypes.bfloat16
    x = np.asarray(x)
    W_base = np.asarray(W_base)
    b_base = np.asarray(b_base)
    W_router = np.asarray(W_router)
    A = np.asarray(A)
    B = np.asarray(B)
    x_flat = np.ascontiguousarray(x, dtype=np.float32).reshape(-1, D_IN)

    wt = W_base.astype(np.float32, copy=False).T            # [d_in, d_out]
    # wto[o, p, k, c] = W^T[k*128 + p, o*512 + c]  (partition-first pack)
    wto = np.ascontiguousarray(
        wt.reshape(K_TILES, 128, O_TILES, 512)
        .transpose(2, 1, 0, 3)
        .astype(bf16)
    )
    acat = A.astype(np.float32, copy=False).reshape(J, D_IN)
    at = acat.T.reshape(K_TILES, 128, J).transpose(1, 0, 2)  # [p, k, j]
    wrt = (
        W_router.astype(np.float32, copy=False)
        .T.reshape(K_TILES, 128, N_EXP)
        .transpose(1, 0, 2)
    )
    arhm = np.ascontiguousarray(
        np.concatenate([at, wrt], axis=2).astype(bf16)
    )
    bft = np.ascontiguousarray(
        (SCALING * B.astype(np.float32, copy=False).transpose(0, 2, 1)
         .reshape(J, D_OUT)).astype(bf16)
    )
    bvec = np.ascontiguousarray(b_base, dtype=np.float32)
    identb = np.eye(128, dtype=np.float32).astype(bf16)

    in_maps = []
    for c in range(N_CORES):
        shard = x_flat[c * TOK:(c + 1) * TOK]               # [1024, 4096]
        # xt[chunk, p, kk, t] = x^T[(chunk*4 + kk)*128 + p, t]
        xt = np.ascontiguousarray(
            shard.T.astype(bf16)
            .reshape(8, 4, 128, TOK)
            .transpose(0, 2, 1, 3)
        )
        in_maps.append({
            "xt": xt, "wto": wto, "arhm": arhm, "bft": bft,
            "bvec": bvec, "identb": identb,
        })
    return in_maps


def _run(in_maps, trace=False, **kw):
    nc = _build_program()
    return run_bass_kernel_spmd(
        nc, in_maps, core_ids=list(range(N_CORES)), trace=trace, **kw
    )


def kernel(x, W_base, b_base, W_router, A, B):
    orig_shape = np.asarray(x).shape
    in_maps = _prep_inputs(x, W_base, b_base, W_router, A, B)
    res = _run(in_maps)
    shards = [res.results[c]["out"] for c in range(N_CORES)]
    out = np.concatenate(shards, axis=0)
    return out.reshape(*orig_shape[:-1], D_OUT).astype(np.float32, copy=False)

